# revision 1
# baseline (speedup 1.0000x reference)
"""Causal self-attention on 8 NeuronCores (Bass/Tile, fp32r matmuls).

Sharding: tensor-parallel over heads x data-parallel over batch.
  core c -> batch b = c//4, heads 4g..4g+3 where g = c%4.
Each core computes q,k,v for its 4 heads (over its batch's 2048 tokens),
causal softmax attention in transposed-score layout [k, q] (denominator via
an extra ones-column on v), and the partial output projection over its 256
head-channels. Host sums the 4 partials per batch and adds b_proj.

Matmuls run as float32r (full PE rate at N>=256, ~1e-4 relative rounding);
the attention probabilities p and values v are bf16 (DVE 2x/4x modes; the
softmax numerator and denominator use the same rounded p, so the error
largely cancels). The 1/sqrt(d) scale is folded into W_k/b_k on the host.

The per-512-token stripes are emitted interleaved (qkv stripe ti, then
attention stripe qi=ti) so the Tile scheduler overlaps PE-heavy projection
work with ACT-heavy softmax work; the output projection is emitted last so
its PE work fills the ACT-bound tail of the late (long) attention stripes.
Diagonal score blocks are narrowed to skip fully-masked columns.
"""

import os
import sys

for _p in ("/opt/trn_rl_repo", "/opt/pypackages"):
    if os.path.isdir(_p) and _p not in sys.path:
        sys.path.append(_p)

import numpy as np

import concourse.bass as bass
import concourse.tile as tile
import concourse.mybir as mybir
from concourse import bacc
from concourse.bass_utils import run_bass_kernel_spmd

B, T, C = 2, 2048, 1024
H = 16            # total heads
D = 64            # head dim
HPC = 4           # heads per core
CH = HPC * D      # 256 channels per core
N_CORES = 8

f32 = mybir.dt.float32
f32r = mybir.dt.float32r
bf16 = mybir.dt.bfloat16
ts = bass.ts

_COMPILED = None


def _build():
    nc = bacc.Bacc("TRN2", target_bir_lowering=False, debug=False,
                   num_devices=N_CORES)

    xT = nc.dram_tensor("xT", [C, T], f32, kind="ExternalInput").ap()
    wt = nc.dram_tensor("wt", [C, 3 * CH], f32, kind="ExternalInput").ap()
    wpt = nc.dram_tensor("wpt", [CH, C], f32, kind="ExternalInput").ap()
    bqk = nc.dram_tensor("bqk", [128, 4], f32, kind="ExternalInput").ap()
    bvb = nc.dram_tensor("bvb", [128, CH], f32, kind="ExternalInput").ap()
    Sm = nc.dram_tensor("Sm", [128, 1024], f32, kind="ExternalInput").ap()
    out = nc.dram_tensor("out_partial", [T, C], f32, kind="ExternalOutput").ap()

    NT512 = T // 512          # 4   512-token stripes
    NT128 = T // 128          # 16  128-token tiles
    NC128 = C // 128          # 8   contraction tiles

    with tile.TileContext(nc) as tc:
        with tc.tile_pool(name="consts", bufs=1) as consts, \
             tc.tile_pool(name="qkv", bufs=1) as qkv, \
             tc.tile_pool(name="xp", bufs=3) as xp, \
             tc.tile_pool(name="pp", bufs=8) as pp, \
             tc.tile_pool(name="op", bufs=6) as op, \
             tc.tile_pool(name="small", bufs=4) as small, \
             tc.tile_pool(name="ps_big", bufs=2, space="PSUM") as ps_big, \
             tc.tile_pool(name="ps_s", bufs=3, space="PSUM") as ps_s, \
             tc.tile_pool(name="ps_y", bufs=1, space="PSUM") as ps_y, \
             tc.tile_pool(name="ps_o", bufs=2, space="PSUM") as ps_o:

            # ---- constants; DMA emission order is chosen so the first
            #      qk matmul chains of stripe 0 can start as early as
            #      possible: interleave xt(0)[ci] with the qk half of
            #      wt[ci], defer the v-half / masks / wpt ----
            xT_r = xT.rearrange("(o p) t -> p o t", p=128).bitcast(f32r)
            wt_r = wt.rearrange("(o p) f -> p o f", p=128).bitcast(f32r)
            wt_sb = consts.tile([128, NC128, 3 * CH], f32r)
            xt0 = xp.tile([128, NC128, 512], f32r, tag="xt")
            for ci in range(NC128):
                nc.sync.dma_start(xt0[:, ci], xT_r[:, ci, ts(0, 512)])
                nc.sync.dma_start(wt_sb[:, ci, :512], wt_r[:, ci, :512])
            bqk_sb = consts.tile([128, 4], f32)
            nc.sync.dma_start(bqk_sb[:], bqk)
            for ci in range(NC128):
                nc.sync.dma_start(wt_sb[:, ci, 512:], wt_r[:, ci, 512:])
            bvb_sb = consts.tile([128, CH], f32)
            nc.sync.dma_start(bvb_sb[:], bvb)
            S_f = consts.tile([128, 1024], f32)
            nc.sync.dma_start(S_f[:], Sm)
            S_sb = consts.tile([128, 1024], bf16)
            nc.vector.tensor_copy(S_sb[:], S_f[:])

            onecol_f = consts.tile([128, 1], f32)
            nc.vector.memset(onecol_f[:], 1.0)

            # ---- persistent activations ----
            qT = qkv.tile([128, 2, T], f32r)      # [2h*64, slab, t]
            kT = qkv.tile([128, 2, T], f32r)
            vaug = qkv.tile([128, NT128, HPC, D + 1], bf16)  # [t128, ti, h, d|1]
            yT = qkv.tile([128, 2, T], f32r)

            for h in range(HPC):
                nc.vector.tensor_copy(
                    vaug[:, :, h, D:D + 1],
                    onecol_f[:].to_broadcast([128, NT128, 1]))

            for ti in range(NT512):
                # ---------- QKV projection for stripe ti ----------
                if ti == 0:
                    xt = xt0
                else:
                    xt = xp.tile([128, NC128, 512], f32r, tag="xt")
                    for ci in range(NC128):
                        nc.sync.dma_start(xt[:, ci], xT_r[:, ci, ts(ti, 512)])
                for fj in range(4):          # q0 q1 k0 k1
                    ps = ps_big.tile([128, 512], f32, tag="big")
                    for ci in range(NC128):
                        nc.tensor.matmul(
                            ps[:], wt_sb[:, ci, ts(fj, 128)], xt[:, ci, :],
                            start=(ci == 0), stop=(ci == NC128 - 1))
                    dest = qT if fj < 2 else kT
                    nc.vector.tensor_add(
                        out=dest[:, fj % 2, ts(ti, 512)], in0=ps[:],
                        in1=bqk_sb[:, fj:fj + 1].to_broadcast([128, 512]))
                for tj in range(4):
                    pv = ps_big.tile([128, 512], f32, tag="big")
                    for ci in range(NC128):
                        nc.tensor.matmul(
                            pv[:, :CH], xt[:, ci, ts(tj, 128)],
                            wt_sb[:, ci, 512:512 + CH],
                            start=(ci == 0), stop=(ci == NC128 - 1))
                    for h in range(HPC):
                        nc.vector.tensor_add(
                            out=vaug[:, 4 * ti + tj, h, 0:D],
                            in0=pv[:, ts(h, D)],
                            in1=bvb_sb[:, ts(h, D)])

                # ---------- attention stripe qi = ti ----------
                qi = ti
                nk = 4 * qi + 4
                for h in range(HPC):
                    hp, hs = (h % 2) * D, h // 2
                    py = ps_y.tile([D + 1, 512], f32)
                    for ki in range(nk):
                        j = ki - 4 * qi
                        # columns qq < 128*j of this stripe are fully masked
                        q0 = max(0, 128 * j)
                        w = 512 - q0
                        psc = ps_s.tile([128, 512], f32)
                        nc.tensor.matmul(
                            psc[:, q0:],
                            kT[hp:hp + D, hs, ts(ki, 128)],
                            qT[hp:hp + D, hs, bass.ds(512 * qi + q0, w)],
                            start=True, stop=True)
                        p = pp.tile([128, 512], bf16)
                        nc.scalar.activation(
                            p[:, q0:], psc[:, q0:],
                            mybir.ActivationFunctionType.Exp)
                        if j >= 0:  # partial 128 columns need the causal mask
                            nc.vector.tensor_mul(
                                out=p[:, q0:q0 + 128], in0=p[:, q0:q0 + 128],
                                in1=S_sb[:, 384:512])
                        nc.tensor.matmul(
                            py[:, q0:], vaug[:, ki, h, :], p[:, q0:],
                            start=(ki == 0), stop=(ki == nk - 1))
                    # normalize: yT = py[:D] * (1/py[D]) broadcast over d
                    rec = small.tile([1, 512], f32, tag="rec")
                    nc.vector.reciprocal(rec[:], py[D:D + 1, :])
                    bc = small.tile([D, 512], f32, tag="bc")
                    nc.gpsimd.partition_broadcast(bc[:], rec[:], channels=D)
                    nc.vector.tensor_mul(
                        out=yT[hp:hp + D, hs, ts(qi, 512)],
                        in0=py[0:D, :], in1=bc[:])

            wpt_sb = consts.tile([128, 2, C], f32r)
            nc.sync.dma_start(
                wpt_sb[:], wpt.rearrange("(s p) o -> p s o", p=128).bitcast(f32r))

            # ---------- output projection (emitted last so its PE work
            #            fills the ACT-bound tail of late attention stripes) --
            for tg in range(NT128):
                for oi in range(2):
                    po = ps_o.tile([128, 512], f32, tag="po")
                    for s in range(2):
                        nc.tensor.matmul(
                            po[:], yT[:, s, ts(tg, 128)],
                            wpt_sb[:, s, ts(oi, 512)],
                            start=(s == 0), stop=(s == 1))
                    ot = op.tile([128, 512], f32)
                    nc.vector.tensor_copy(ot[:], po[:])
                    nc.sync.dma_start(
                        out[ts(tg, 128), ts(oi, 512)], ot[:])

    nc.compile()
    return nc


def _get_compiled():
    global _COMPILED
    if _COMPILED is None:
        _COMPILED = _build()
    return _COMPILED


def _host_prep(x, W_attn, b_attn, W_proj, b_proj):
    scale = 1.0 / np.sqrt(np.float32(D))
    xTb = [np.ascontiguousarray(x[b].T).astype(np.float32) for b in range(B)]
    Sm = (np.arange(1024, dtype=np.int32)[None, :]
          >= (np.arange(128, dtype=np.int32)[:, None] + 384)).astype(np.float32)
    in_maps = []
    for c in range(N_CORES):
        b, g = divmod(c, 4)
        ch = slice(CH * g, CH * (g + 1))
        Wq = W_attn[ch]
        Wk = W_attn[C:][ch] * scale
        Wv = W_attn[2 * C:][ch]
        wt_c = np.ascontiguousarray(
            np.concatenate([Wq, Wk, Wv], axis=0).T).astype(np.float32)
        bq = b_attn[ch]
        bk = b_attn[C:][ch] * scale
        bv = b_attn[2 * C:][ch]
        bqk_c = np.ascontiguousarray(
            np.concatenate([bq, bk]).reshape(4, 128).T).astype(np.float32)
        bvb_c = np.ascontiguousarray(
            np.broadcast_to(bv[None, :], (128, CH))).astype(np.float32)
        wpt_c = np.ascontiguousarray(W_proj[:, ch].T).astype(np.float32)
        in_maps.append({
            "xT": xTb[b],
            "wt": wt_c,
            "wpt": wpt_c,
            "bqk": bqk_c,
            "bvb": bvb_c,
            "Sm": Sm,
        })
    return in_maps


def kernel(x, W_attn, b_attn, W_proj, b_proj):
    x = np.asarray(x, dtype=np.float32)
    W_attn = np.asarray(W_attn, dtype=np.float32)
    b_attn = np.asarray(b_attn, dtype=np.float32)
    W_proj = np.asarray(W_proj, dtype=np.float32)
    b_proj = np.asarray(b_proj, dtype=np.float32)

    nc = _get_compiled()
    in_maps = _host_prep(x, W_attn, b_attn, W_proj, b_proj)
    res = run_bass_kernel_spmd(nc, in_maps, core_ids=list(range(N_CORES)))

    out = np.empty((B, T, C), dtype=np.float32)
    for b in range(B):
        acc = res.results[4 * b]["out_partial"].copy()
        for g in range(1, 4):
            acc += res.results[4 * b + g]["out_partial"]
        out[b] = acc + b_proj
    return out



# revision 23
# speedup vs baseline: 1.0844x; 1.0844x over previous
"""Causal self-attention on 8 NeuronCores (Bass/Tile, bf16 matmuls).

Sharding: tensor-parallel over heads x data-parallel over batch.
  core c -> batch b = c//4, heads 4g..4g+3 where g = c%4.
Each core computes q,k,v for its 4 heads (over its batch's 2048 tokens),
causal softmax attention in transposed-score layout [k, q] (denominator via
an extra ones-column on v), and the partial output projection over its 256
head-channels. Host sums the 4 partials per batch and adds b_proj plus the
v-bias term W_proj @ b_v (the v bias passes through softmax exactly, so it
is folded out of the device program entirely).

All matmul operands are bf16 (host pre-converts x and the weights; 1/sqrt(d)
is folded into W_k/b_k). PSUM accumulation stays fp32. Softmax exp runs on
ACT in PAIRS of score tiles (one activation over 2 PSUM banks) to halve the
per-instruction access bubbles; causal masking multiplies the diagonal
128-col blocks by a lower-triangular bf16 mask on DVE.

Emission is software-pipelined at block granularity: iteration i interleaves
attention stripe i-1 (ACT/DVE-heavy) with the QKV projection of stripe i and
the output projection of stripe i-2 (PE-heavy), so the PE stream always has
matmul work while ACT digests exp batches. v-tile and output-staging copies
run on GPSIMD to keep DVE free for the softmax-critical mask/normalize ops.
"""

import os
import sys

for _p in ("/opt/trn_rl_repo", "/opt/pypackages"):
    if os.path.isdir(_p) and _p not in sys.path:
        sys.path.append(_p)

import numpy as np
import ml_dtypes

import concourse.bass as bass
import concourse.tile as tile
import concourse.mybir as mybir
from concourse import bacc
from concourse.bass_utils import run_bass_kernel_spmd

B, T, C = 2, 2048, 1024
H = 16            # total heads
D = 64            # head dim
HPC = 4           # heads per core
CH = HPC * D      # 256 channels per core
N_CORES = 8

f32 = mybir.dt.float32
bf16 = mybir.dt.bfloat16
ts = bass.ts
ds = bass.ds
EXP = mybir.ActivationFunctionType.Exp

NT512 = T // 512          # 4   512-token stripes
NT128 = T // 128          # 16  128-token tiles
NC128 = C // 128          # 8   contraction tiles

_COMPILED = None


def _build():
    nc = bacc.Bacc("TRN2", target_bir_lowering=False, debug=False,
                   num_devices=N_CORES)

    xT = nc.dram_tensor("xT", [C, T], bf16, kind="ExternalInput").ap()
    wt = nc.dram_tensor("wt", [C, 3 * CH], bf16, kind="ExternalInput").ap()
    wpt = nc.dram_tensor("wpt", [CH, C], bf16, kind="ExternalInput").ap()
    bqk = nc.dram_tensor("bqk", [128, 4], f32, kind="ExternalInput").ap()
    Sm = nc.dram_tensor("Sm", [128, 128], bf16, kind="ExternalInput").ap()
    Idm = nc.dram_tensor("Idm", [128, 128], bf16, kind="ExternalInput").ap()
    out = nc.dram_tensor("out_partial", [T, C], bf16, kind="ExternalOutput").ap()

    xT_r = xT.rearrange("(o p) t -> p o t", p=128)
    wt_r = wt.rearrange("(o p) f -> p o f", p=128)

    with tile.TileContext(nc) as tc:
        with tc.tile_pool(name="consts", bufs=1) as consts, \
             tc.tile_pool(name="qkv", bufs=1) as qkv, \
             tc.tile_pool(name="xp", bufs=2) as xp, \
             tc.tile_pool(name="pp", bufs=4) as pp, \
             tc.tile_pool(name="op", bufs=3) as op, \
             tc.tile_pool(name="small", bufs=4) as small, \
             tc.tile_pool(name="ps_qkv", bufs=2, space="PSUM") as ps_qkv, \
             tc.tile_pool(name="ps_s", bufs=2, space="PSUM") as ps_s, \
             tc.tile_pool(name="ps_y", bufs=2, space="PSUM") as ps_y:

            # ---- persistent tiles ----
            wt_sb = consts.tile([128, NC128, 3 * CH], bf16)
            wpt_sb = consts.tile([128, 2, C], bf16)
            bqk_sb = consts.tile([128, 4], f32)
            S_sb = consts.tile([128, 128], bf16)
            I_sb = consts.tile([128, 128], bf16)
            hs = consts.tile([128, 4, C], bf16)   # stripe-3 slab-0 half-sums
            # per-stripe tiles (separate tensors so the tile framework's
            # name-level dependency tracking never sees false stripe-to-
            # stripe hazards between attention, QKV and proj work)
            qTs = [qkv.tile([128, 2, 512], bf16, name=f"qT{i}")
                   for i in range(NT512)]
            kTs = [qkv.tile([128, 2, 512], bf16, name=f"kT{i}")
                   for i in range(NT512)]
            vas = [qkv.tile([128, 4, HPC, D + 1], bf16, name=f"va{i}")
                   for i in range(NT512)]
            yTs = [qkv.tile([128, 2, 512], bf16, name=f"yT{i}")
                   for i in range(NT512)]
            out_r = out.rearrange("(o p) c -> p o c", p=128)

            # ---- startup DMAs: stripe-0 x interleaved with the qk half of
            #      the weights (2-ci chunks amortize the per-DMA fixed cost
            #      while still drip-feeding the first accumulation chains) ----
            xts = [xp.tile([128, NC128, 512], bf16, tag="xt", name=f"xt{i}")
                   for i in range(2)]
            for cs in (slice(0, 1), slice(1, 2), slice(2, 3), slice(3, 5),
                       slice(5, 7), slice(7, 8)):
                nc.sync.dma_start(xts[0][:, cs, :], xT_r[:, cs, ts(0, 512)])
                nc.scalar.dma_start(wt_sb[:, cs, :512], wt_r[:, cs, :512])
            nc.gpsimd.dma_start(bqk_sb[:], bqk)
            for cp in range(2):
                cs = slice(4 * cp, 4 * cp + 4)
                nc.scalar.dma_start(wt_sb[:, cs, 512:], wt_r[:, cs, 512:])
            nc.gpsimd.dma_start(S_sb[:], Sm)
            nc.gpsimd.dma_start(I_sb[:], Idm)

            onecol_f = consts.tile([128, 1], f32)
            nc.vector.memset(onecol_f[:], 1.0)
            onerow_f = consts.tile([1, D], f32)
            nc.vector.memset(onerow_f[:], 1.0)
            for i in range(NT512):
                for h in range(HPC):
                    nc.vector.tensor_copy(
                        vas[i][:, :, h, D:D + 1],
                        onecol_f[:].to_broadcast([128, 4, 1]))

            # ---------- emission units ----------
            def qkv_units(ti):
                xt = xts[ti % 2]
                units = []
                for fj in range(4):          # q0 q1 k0 k1
                    def unit(fj=fj, xt=xt, ti=ti):
                        ps = ps_qkv.tile([128, 512], f32, tag="big", name="ps")
                        for ci in range(NC128):
                            nc.tensor.matmul(
                                ps[:], wt_sb[:, ci, ts(fj, 128)], xt[:, ci, :],
                                start=(ci == 0), stop=(ci == NC128 - 1))
                        dest = qTs[ti] if fj < 2 else kTs[ti]
                        nc.vector.tensor_add(
                            out=dest[:, fj % 2, :], in0=ps[:],
                            in1=bqk_sb[:, fj:fj + 1].to_broadcast([128, 512]))
                    units.append(unit)
                for tj in range(4):
                    def unit(tj=tj, xt=xt, ti=ti):
                        pv = ps_qkv.tile([128, 512], f32, tag="big", name="pv")
                        for ci in range(NC128):
                            nc.tensor.matmul(
                                pv[:, :CH], xt[:, ci, ts(tj, 128)],
                                wt_sb[:, ci, 512:512 + CH],
                                start=(ci == 0), stop=(ci == NC128 - 1))
                        nc.vector.tensor_copy(
                            vas[ti][:, tj, :, 0:D],
                            pv[:, :CH].rearrange("p (a b) -> p a b", b=D))
                    units.append(unit)
                return units

            def attn_units(qi, heads=range(HPC)):
                units = []
                nk = 4 * qi + 4
                for h in heads:
                    hp, hs_ = (h % 2) * D, h // 2
                    state = {}
                    for kp in range(nk // 2):
                        def unit(h=h, hp=hp, hs_=hs_, kp=kp, qi=qi, nk=nk,
                                 state=state):
                            if kp == 0:
                                state["py"] = ps_y.tile([D + 1, 512], f32,
                                                        tag="py", name="py")
                            py = state["py"]
                            psc = ps_s.tile([128, 2, 512], f32, tag="sc", name="psc")
                            p = pp.tile([128, 2, 512], bf16, tag="p", name="p")
                            q0s = []
                            for sb in range(2):
                                ki = 2 * kp + sb
                                j = ki - 4 * qi
                                q0 = max(0, 128 * j)
                                q0s.append(q0)
                                nc.tensor.matmul(
                                    psc[:, sb, q0:],
                                    kTs[ki // 4][hp:hp + D, hs_, ts(ki % 4, 128)],
                                    qTs[qi][hp:hp + D, hs_, ds(q0, 512 - q0)],
                                    start=True, stop=True)
                            q0p = q0s[0]
                            nc.scalar.activation(
                                p[:, :, q0p:], psc[:, :, q0p:], EXP)
                            for sb in range(2):
                                q0 = q0s[sb]
                                if 2 * kp + sb >= 4 * qi:  # diagonal block
                                    nc.vector.tensor_mul(
                                        out=p[:, sb, q0:q0 + 128],
                                        in0=p[:, sb, q0:q0 + 128],
                                        in1=S_sb[:])
                            for sb in range(2):
                                ki = 2 * kp + sb
                                q0 = q0s[sb]
                                nc.tensor.matmul(
                                    py[:, q0:], vas[ki // 4][:, ki % 4, h, :],
                                    p[:, sb, q0:],
                                    start=(ki == 0), stop=(ki == nk - 1))
                        units.append(unit)

                    def norm(h=h, hp=hp, hs_=hs_, qi=qi, state=state):
                        py = state["py"]
                        if qi == 3 and h == 3:
                            # tail-critical: pipeline the normalize in halves
                            for cl in range(2):
                                cs = ds(256 * cl, 256)
                                rc = small.tile([1, 256], f32, tag="rec",
                                                name="rc")
                                nc.vector.reciprocal(rc[:], py[D:D + 1, cs])
                                bh = small.tile([D, 256], f32, tag="bc",
                                                name="bh")
                                nc.gpsimd.partition_broadcast(bh[:], rc[:],
                                                              channels=D)
                                nc.vector.tensor_mul(
                                    out=yTs[qi][hp:hp + D, hs_, cs],
                                    in0=py[0:D, cs], in1=bh[:])
                            return
                        rec = small.tile([1, 512], f32, tag="rec", name="rec")
                        nc.vector.reciprocal(rec[:], py[D:D + 1, :])
                        bc = small.tile([D, 512], f32, tag="bc", name="bc")
                        nc.gpsimd.partition_broadcast(bc[:], rec[:],
                                                      channels=D)
                        nc.vector.tensor_mul(
                            out=yTs[qi][hp:hp + D, hs_, :],
                            in0=py[0:D, :], in1=bc[:])
                    units.append(norm)
                return units

            COPY = mybir.ActivationFunctionType.Copy

            def proj_units(si, engines, dma_splits=1):
                units = []
                state = {}
                order = [(tgl, oi) for tgl in range(4) for oi in range(2)]
                nu = len(order)
                for u_i, (tgl, oi) in enumerate(order):
                    def unit(tgl=tgl, oi=oi, u_i=u_i, si=si, state=state):
                        if u_i == 0:
                            state["otS"] = op.tile([128, 4, C], bf16,
                                                   tag="otS", name="otS")
                        po = ps_qkv.tile([128, 512], f32, tag="big", name="po")
                        for s in range(2):
                            nc.tensor.matmul(
                                po[:], yTs[si][:, s, ts(tgl, 128)],
                                wpt_sb[:, s, ts(oi, 512)],
                                start=(s == 0), stop=(s == 1))
                        eng = engines[u_i % len(engines)]
                        dst = state["otS"][:, tgl, ts(oi, 512)]
                        if eng == "act":
                            nc.scalar.activation(dst, po[:], COPY)
                        else:
                            nc.vector.tensor_copy(dst, po[:])
                        done = u_i + 1
                        per = nu // dma_splits
                        if done % per == 0:
                            lo = (done - per) // 2
                            hi = done // 2
                            nc.sync.dma_start(
                                out_r[:, 4 * si + lo:4 * si + hi, :],
                                state["otS"][:, lo:hi, :])
                    units.append(unit)
                return units

            def proj3a_units(engines):
                # stripe-3 slab-0 (heads 0,1) partial proj into SBUF, so the
                # post-attention tail only runs the slab-1 half
                units = []
                for u_i, (tgl, oi) in enumerate(
                        (tgl, oi) for tgl in range(4) for oi in range(2)):
                    def unit(tgl=tgl, oi=oi, u_i=u_i):
                        po = ps_qkv.tile([128, 512], f32, tag="big", name="po")
                        nc.tensor.matmul(
                            po[:], yTs[3][:, 0, ts(tgl, 128)],
                            wpt_sb[:, 0, ts(oi, 512)], start=True, stop=True)
                        dst = hs[:, tgl, ts(oi, 512)]
                        if engines[u_i % len(engines)] == "act":
                            nc.scalar.activation(dst, po[:], COPY)
                        else:
                            nc.vector.tensor_copy(dst, po[:])
                    units.append(unit)
                return units

            def proj3b_units(engines):
                units = []
                state = {}
                for u_i, (tgl, oi) in enumerate(
                        (tgl, oi) for tgl in range(4) for oi in range(2)):
                    def unit(tgl=tgl, oi=oi, u_i=u_i, state=state):
                        if u_i == 0:
                            state["otS"] = op.tile([128, 4, C], bf16,
                                                   tag="otS", name="otS")
                        if u_i % 2 == 0:
                            po = ps_qkv.tile([128, 512], f32, tag="big",
                                             name="po")
                        else:
                            pot = ps_s.tile([128, 2, 512], f32, tag="sc",
                                            name="pot")
                            po = pot[:, 0, :]
                        nc.tensor.matmul(
                            po[:], I_sb[:], hs[:, tgl, ts(oi, 512)],
                            start=True, stop=False)
                        nc.tensor.matmul(
                            po[:], yTs[3][:, 1, ts(tgl, 128)],
                            wpt_sb[:, 1, ts(oi, 512)], start=False, stop=True)
                        dst = state["otS"][:, tgl, ts(oi, 512)]
                        if engines[u_i % len(engines)] == "act":
                            nc.scalar.activation(dst, po[:], COPY)
                        else:
                            nc.vector.tensor_copy(dst, po[:])
                        if (u_i + 1) % 2 == 0:
                            lo = (u_i + 1) // 2 - 1
                            nc.sync.dma_start(
                                out_r[:, 12 + lo:13 + lo, :],
                                state["otS"][:, lo:lo + 1, :])
                    units.append(unit)
                return units

            def interleave(primary, fillers):
                n, m = len(primary), len(fillers)
                fi, acc = 0, 0.0
                if m:
                    fillers[0]()
                    fi = 1
                for u in primary:
                    u()
                    acc += m / n
                    while fi < m and fi < int(acc + 1e-9) + 1:
                        fillers[fi]()
                        fi += 1
                while fi < m:
                    fillers[fi]()
                    fi += 1

            # iter 0: QKV stripe 0 alone; prefetch x stripe 1, then wpt
            nc.sync.dma_start(xts[1][:, :, :], xT_r[:, :, ts(1, 512)])
            nc.sync.dma_start(
                wpt_sb[:], wpt.rearrange("(s p) o -> p s o", p=128))
            for u in qkv_units(0):
                u()

            # steady iters: attention i-1 + QKV i (proj is all deferred to
            # the late, otherwise ACT-bound iterations)
            for i in range(1, 4):
                if i < 3:
                    xt_next = xts[(i + 1) % 2]
                    nc.sync.dma_start(xt_next[:, :, :],
                                      xT_r[:, :, ts(i + 1, 512)])
                interleave(attn_units(i - 1), qkv_units(i))
                if i == 3:
                    # stripe-3 head 0 pulled forward (qkv 3 is complete by
                    # late iter 3) so the final iteration stays PE-bound
                    interleave(attn_units(3, heads=[0]),
                               proj_units(0, ["dve"], dma_splits=2))

            # stripe-3 head 1 + proj 1; then heads 2-3 with proj 2 and the
            # slab-0 half of proj 3 (h0/h1 normalized by then) as PE fill
            interleave(attn_units(3, heads=[1]),
                       proj_units(1, ["dve"], dma_splits=2))
            interleave(attn_units(3, heads=[2, 3]),
                       proj_units(2, ["dve"], dma_splits=2)
                       + proj3a_units(["dve"]))
            # tail: only the slab-1 half remains (PE re-injects the slab-0
            # half via an identity matmul); copies fan across idle engines
            for u in proj3b_units(["act", "dve"]):
                u()

    nc.compile()
    return nc


def _get_compiled():
    global _COMPILED
    if _COMPILED is None:
        _COMPILED = _build()
    return _COMPILED


def _host_prep(x, W_attn, b_attn, W_proj, b_proj):
    scale = 1.0 / np.sqrt(np.float32(D))
    b16 = ml_dtypes.bfloat16
    xTb = [np.ascontiguousarray(x[b].T).astype(b16) for b in range(B)]
    Sm = (np.arange(128, dtype=np.int32)[None, :]
          >= np.arange(128, dtype=np.int32)[:, None]).astype(b16)
    Idm = np.eye(128, dtype=np.float32).astype(b16)
    in_maps = []
    for c in range(N_CORES):
        b, g = divmod(c, 4)
        ch = slice(CH * g, CH * (g + 1))
        Wq = W_attn[ch]
        Wk = W_attn[C:][ch] * scale
        Wv = W_attn[2 * C:][ch]
        wt_c = np.ascontiguousarray(
            np.concatenate([Wq, Wk, Wv], axis=0).T).astype(b16)
        bq = b_attn[ch]
        bk = b_attn[C:][ch] * scale
        bqk_c = np.ascontiguousarray(
            np.concatenate([bq, bk]).reshape(4, 128).T).astype(np.float32)
        wpt_c = np.ascontiguousarray(W_proj[:, ch].T).astype(b16)
        in_maps.append({
            "xT": xTb[b],
            "wt": wt_c,
            "wpt": wpt_c,
            "bqk": bqk_c,
            "Sm": Sm,
            "Idm": Idm,
        })
    return in_maps


def kernel(x, W_attn, b_attn, W_proj, b_proj):
    x = np.asarray(x, dtype=np.float32)
    W_attn = np.asarray(W_attn, dtype=np.float32)
    b_attn = np.asarray(b_attn, dtype=np.float32)
    W_proj = np.asarray(W_proj, dtype=np.float32)
    b_proj = np.asarray(b_proj, dtype=np.float32)

    nc = _get_compiled()
    in_maps = _host_prep(x, W_attn, b_attn, W_proj, b_proj)
    res = run_bass_kernel_spmd(nc, in_maps, core_ids=list(range(N_CORES)))

    # v-bias passes through softmax exactly: y = y_nobias + b_v, so its
    # output-projection contribution W_proj @ b_v is added here instead
    # of on the device.
    bv = b_attn[2 * C:]
    b_eff = (b_proj + W_proj.astype(np.float64) @ bv.astype(np.float64)
             ).astype(np.float32)

    out = np.empty((B, T, C), dtype=np.float32)
    for b in range(B):
        acc = res.results[4 * b]["out_partial"].astype(np.float32)
        for g in range(1, 4):
            acc += res.results[4 * b + g]["out_partial"].astype(np.float32)
        out[b] = acc + b_eff
    return out


# revision 36
# speedup vs baseline: 1.1928x; 1.0999x over previous
"""Causal self-attention on 8 NeuronCores (Bass/Tile, bf16 matmuls).

Sharding: tensor-parallel over heads x data-parallel over batch.
  core c -> batch b = c//4, heads 4g..4g+3 where g = c%4.
Each core computes q,k,v for its 4 heads (over its batch's 2048 tokens),
causal softmax attention in transposed-score layout [k, q] (denominator via
an extra ones-column on v), and the partial output projection over its 256
head-channels. Host sums the 4 partials per batch and adds b_proj plus the
v-bias term W_proj @ b_v (the v bias passes through softmax exactly, so it
is folded out of the device program entirely).

All matmul operands are bf16 (host pre-converts x and the weights; 1/sqrt(d)
is folded into W_k/b_k). PSUM accumulation stays fp32. Softmax exp runs on
ACT in PAIRS of score tiles (one activation over 2 PSUM banks) to halve the
per-instruction access bubbles; causal masking multiplies the diagonal
128-col blocks by a lower-triangular bf16 mask on DVE.

Emission is software-pipelined at block granularity: iteration i interleaves
attention stripe i-1 (ACT/DVE-heavy) with the QKV projection of stripe i and
the output projection of stripe i-2 (PE-heavy), so the PE stream always has
matmul work while ACT digests exp batches. v-tile and output-staging copies
run on GPSIMD to keep DVE free for the softmax-critical mask/normalize ops.
"""

import os
import sys

for _p in ("/opt/trn_rl_repo", "/opt/pypackages"):
    if os.path.isdir(_p) and _p not in sys.path:
        sys.path.append(_p)

import numpy as np
import ml_dtypes

import concourse.bass as bass
import concourse.tile as tile
import concourse.mybir as mybir
from concourse import bacc
from concourse.bass_utils import run_bass_kernel_spmd

B, T, C = 2, 2048, 1024
H = 16            # total heads
D = 64            # head dim
HPC = 4           # heads per core
CH = HPC * D      # 256 channels per core
N_CORES = 8

f32 = mybir.dt.float32
bf16 = mybir.dt.bfloat16
ts = bass.ts
ds = bass.ds
EXP = mybir.ActivationFunctionType.Exp

NT512 = T // 512          # 4   512-token stripes
NT128 = T // 128          # 16  128-token tiles
NC128 = C // 128          # 8   contraction tiles

_COMPILED = None


def _build():
    nc = bacc.Bacc("TRN2", target_bir_lowering=False, debug=False,
                   num_devices=N_CORES)

    xT = nc.dram_tensor("xT", [C, T], bf16, kind="ExternalInput").ap()
    wt = nc.dram_tensor("wt", [C, 3 * CH], bf16, kind="ExternalInput").ap()
    wpt = nc.dram_tensor("wpt", [CH, C], bf16, kind="ExternalInput").ap()
    bqk = nc.dram_tensor("bqk", [128, 4], f32, kind="ExternalInput").ap()
    Sm = nc.dram_tensor("Sm", [128, 128], bf16, kind="ExternalInput").ap()
    Idm = nc.dram_tensor("Idm", [128, 128], bf16, kind="ExternalInput").ap()
    out = nc.dram_tensor("out_partial", [T, C], bf16, kind="ExternalOutput").ap()

    xT_r = xT.rearrange("(o p) t -> p o t", p=128)
    wt_r = wt.rearrange("(o p) f -> p o f", p=128)

    with tile.TileContext(nc) as tc:
        with tc.tile_pool(name="consts", bufs=1) as consts, \
             tc.tile_pool(name="qkv", bufs=1) as qkv, \
             tc.tile_pool(name="xp", bufs=2) as xp, \
             tc.tile_pool(name="pp", bufs=6) as pp, \
             tc.tile_pool(name="op", bufs=4) as op, \
             tc.tile_pool(name="small", bufs=8) as small, \
             tc.tile_pool(name="ps_qkv", bufs=2, space="PSUM") as ps_qkv, \
             tc.tile_pool(name="ps_s", bufs=2, space="PSUM") as ps_s, \
             tc.tile_pool(name="ps_y", bufs=2, space="PSUM") as ps_y:

            # ---- persistent tiles ----
            wt_sb = consts.tile([128, NC128, 3 * CH], bf16)
            wpt_sb = consts.tile([128, 2, C], bf16)
            bqk_sb = consts.tile([128, 4], f32)
            S_sb = consts.tile([128, 128], bf16)
            I_sb = consts.tile([128, 128], bf16)
            hs = consts.tile([128, 4, C], bf16)   # stripe-3 slab-0 half-sums
            # per-stripe tiles (separate tensors so the tile framework's
            # name-level dependency tracking never sees false stripe-to-
            # stripe hazards between attention, QKV and proj work)
            # q/k are stored fp8(e4m3) in DoubleRow layout: partition
            # 32*h + (d%32), free dims [i=d//32, t]. The score matmul then
            # runs in DoubleRow perf mode at 0.5 cycles/row.
            fp8 = mybir.dt.float8e4
            qTs = [[qkv.tile([64, 2, 512], fp8, name=f"qT{i}_{s}")
                    for s in range(2)] for i in range(NT512)]
            kTs = [[qkv.tile([64, 2, 512], fp8, name=f"kT{i}_{s}")
                    for s in range(2)] for i in range(NT512)]
            vas = [qkv.tile([128, 4, HPC, D + 1], bf16, name=f"va{i}")
                   for i in range(NT512)]
            yTs = [qkv.tile([128, 2, 512], bf16, name=f"yT{i}")
                   for i in range(NT512)]
            out_r = out.rearrange("(o p) c -> p o c", p=128)

            # ---- startup DMAs: stripe-0 x interleaved with the qk half of
            #      the weights (2-ci chunks amortize the per-DMA fixed cost
            #      while still drip-feeding the first accumulation chains) ----
            xts = [xp.tile([128, NC128, 512], bf16, tag="xt", name=f"xt{i}")
                   for i in range(2)]
            for cs in (slice(0, 1), slice(1, 2), slice(2, 3), slice(3, 5),
                       slice(5, 7), slice(7, 8)):
                nc.sync.dma_start(xts[0][:, cs, :], xT_r[:, cs, ts(0, 512)])
                nc.scalar.dma_start(wt_sb[:, cs, :512], wt_r[:, cs, :512])
            nc.gpsimd.dma_start(bqk_sb[:], bqk)
            for cp in range(2):
                cs = slice(4 * cp, 4 * cp + 4)
                nc.sync.dma_start(wt_sb[:, cs, 512:], wt_r[:, cs, 512:])
            nc.gpsimd.dma_start(S_sb[:], Sm)
            nc.gpsimd.dma_start(I_sb[:], Idm)

            onecol_f = consts.tile([128, 1], f32)
            nc.vector.memset(onecol_f[:], 1.0)
            onerow_f = consts.tile([1, D], f32)
            nc.vector.memset(onerow_f[:], 1.0)
            for i in range(NT512):
                for h in range(HPC):
                    nc.vector.tensor_copy(
                        vas[i][:, :, h, D:D + 1],
                        onecol_f[:].to_broadcast([128, 4, 1]))

            # ---------- emission units ----------
            def qkv_units(ti):
                # (unit, pe_fill_ns) pairs; chains split into halves so the
                # interleaver can pace PE fill finely
                xt = xts[ti % 2]
                units = []
                for fj in range(4):          # q0 q1 k0 k1
                    state = {}
                    def unit_a(fj=fj, xt=xt, state=state):
                        state["ps"] = ps_qkv.tile([128, 512], f32, tag="big",
                                                  name="ps")
                        for ci in range(4):
                            nc.tensor.matmul(
                                state["ps"][:], wt_sb[:, ci, ts(fj, 128)],
                                xt[:, ci, :], start=(ci == 0), stop=False)
                    def unit_b(fj=fj, xt=xt, ti=ti, state=state):
                        ps = state["ps"]
                        for ci in range(4, NC128):
                            nc.tensor.matmul(
                                ps[:], wt_sb[:, ci, ts(fj, 128)], xt[:, ci, :],
                                start=False, stop=(ci == NC128 - 1))
                        dest = (qTs[ti] if fj < 2 else kTs[ti])[fj % 2]
                        nc.vector.tensor_add(
                            out=dest[:, 0, :], in0=ps[0:64, :],
                            in1=bqk_sb[0:64, fj:fj + 1].to_broadcast([64, 512]))
                        nc.vector.tensor_add(
                            out=dest[:, 1, :], in0=ps[64:128, :],
                            in1=bqk_sb[64:128, fj:fj + 1].to_broadcast(
                                [64, 512]))
                    units.append((unit_a, 852))
                    units.append((unit_b, 852))
                for tj in range(4):
                    state = {}
                    def unit_a(tj=tj, xt=xt, ti=ti, state=state):
                        if ti == 0:
                            pvt = ps_s.tile([128, 2, 512], f32, tag="sc",
                                            name="pvt")
                            state["pv"] = pvt[:, 0, :]
                        else:
                            state["pv"] = ps_qkv.tile(
                                [128, 512], f32, tag="big", name="pv")
                        for ci in range(4):
                            nc.tensor.matmul(
                                state["pv"][:, :CH], xt[:, ci, ts(tj, 128)],
                                wt_sb[:, ci, 512:512 + CH],
                                start=(ci == 0), stop=False)
                    def unit_b(tj=tj, xt=xt, ti=ti, state=state):
                        pv = state["pv"]
                        for ci in range(4, NC128):
                            nc.tensor.matmul(
                                pv[:, :CH], xt[:, ci, ts(tj, 128)],
                                wt_sb[:, ci, 512:512 + CH],
                                start=False, stop=(ci == NC128 - 1))
                        nc.vector.tensor_copy(
                            vas[ti][:, tj, :, 0:D],
                            pv[:, :CH].rearrange("p (a b) -> p a b", b=D))
                    units.append((unit_a, 426))
                    units.append((unit_b, 426))
                return units

            def attn_units(qi, heads=range(HPC)):
                units = []
                nk = 4 * qi + 4
                for h in heads:
                    hp, hs_ = (h % 2) * D, h // 2
                    state = {}
                    for kp in range(nk // 2):
                        def unit(h=h, hp=hp, hs_=hs_, kp=kp, qi=qi, nk=nk,
                                 state=state):
                            if kp == 0:
                                state["py"] = ps_y.tile([D + 1, 512], f32,
                                                        tag="py", name="py")
                            py = state["py"]
                            psc = ps_s.tile([128, 2, 512], f32, tag="sc", name="psc")
                            p = pp.tile([128, 2, 512], bf16, tag="p", name="p")
                            q0s = []
                            for sb in range(2):
                                ki = 2 * kp + sb
                                j = ki - 4 * qi
                                q0 = max(0, 128 * j)
                                q0s.append(q0)
                                hl = 32 * (h % 2)
                                nc.tensor.matmul(
                                    psc[:, sb, q0:],
                                    kTs[ki // 4][h // 2][hl:hl + 32, :,
                                                         ts(ki % 4, 128)],
                                    qTs[qi][h // 2][hl:hl + 32, :,
                                                    ds(q0, 512 - q0)],
                                    start=True, stop=True,
                                    perf_mode=mybir.MatmulPerfMode.DoubleRow)
                            q0p = q0s[0]
                            nc.scalar.activation(
                                p[:, :, q0p:], psc[:, :, q0p:], EXP)
                            for sb in range(2):
                                q0 = q0s[sb]
                                if 2 * kp + sb >= 4 * qi:  # diagonal block
                                    nc.vector.tensor_mul(
                                        out=p[:, sb, q0:q0 + 128],
                                        in0=p[:, sb, q0:q0 + 128],
                                        in1=S_sb[:])
                            for sb in range(2):
                                ki = 2 * kp + sb
                                q0 = q0s[sb]
                                nc.tensor.matmul(
                                    py[:, q0:], vas[ki // 4][:, ki % 4, h, :],
                                    p[:, sb, q0:],
                                    start=(ki == 0), stop=(ki == nk - 1))
                        q0p_w = max(0, 128 * (2 * kp - 4 * qi))
                        units.append((unit, int(2 * (512 - q0p_w) * 0.83)
                                      + 370))

                    def norm(h=h, hp=hp, hs_=hs_, qi=qi, state=state):
                        py = state["py"]
                        if qi == 3 and h == 3:
                            # tail-critical: pipeline the normalize in
                            # halves, grouped per op so the in-order
                            # engines overlap
                            NQ, W = 2, 256
                            rcs, bhs = [], []
                            for cl in range(NQ):
                                rc = small.tile([1, W], f32, tag="rec",
                                                name="rc")
                                nc.vector.reciprocal(
                                    rc[:], py[D:D + 1, ds(W * cl, W)])
                                rcs.append(rc)
                            for cl in range(NQ):
                                bh = small.tile([D, W], f32, tag="bc",
                                                name="bh")
                                nc.gpsimd.partition_broadcast(
                                    bh[:], rcs[cl][:], channels=D)
                                bhs.append(bh)
                            for cl in range(NQ):
                                cs = ds(W * cl, W)
                                nc.vector.tensor_mul(
                                    out=yTs[qi][hp:hp + D, hs_, cs],
                                    in0=py[0:D, cs], in1=bhs[cl][:])
                            return
                        rec = small.tile([1, 512], f32, tag="rec", name="rec")
                        nc.vector.reciprocal(rec[:], py[D:D + 1, :])
                        bc = small.tile([D, 512], f32, tag="bc", name="bc")
                        nc.gpsimd.partition_broadcast(bc[:], rec[:],
                                                      channels=D)
                        nc.vector.tensor_mul(
                            out=yTs[qi][hp:hp + D, hs_, :],
                            in0=py[0:D, :], in1=bc[:])
                    units.append((norm, 300))
                return units

            COPY = mybir.ActivationFunctionType.Copy

            def proj_units(si, engines, dma_splits=1):
                units = []
                state = {}
                order = [(tgl, oi) for tgl in range(4) for oi in range(2)]
                nu = len(order)
                for u_i, (tgl, oi) in enumerate(order):
                    def unit(tgl=tgl, oi=oi, u_i=u_i, si=si, state=state):
                        if u_i == 0:
                            state["otS"] = op.tile([128, 4, C], bf16,
                                                   tag="otS", name="otS")
                        po = ps_qkv.tile([128, 512], f32, tag="big", name="po")
                        for s in range(2):
                            nc.tensor.matmul(
                                po[:], yTs[si][:, s, ts(tgl, 128)],
                                wpt_sb[:, s, ts(oi, 512)],
                                start=(s == 0), stop=(s == 1))
                        eng = engines[u_i % len(engines)]
                        dst = state["otS"][:, tgl, ts(oi, 512)]
                        if eng == "act":
                            nc.scalar.activation(dst, po[:], COPY)
                        else:
                            nc.vector.tensor_copy(dst, po[:])
                        done = u_i + 1
                        per = nu // dma_splits
                        if done % per == 0:
                            lo = (done - per) // 2
                            hi = done // 2
                            nc.sync.dma_start(
                                out_r[:, 4 * si + lo:4 * si + hi, :],
                                state["otS"][:, lo:hi, :])
                    units.append((unit, 426))
                return units

            def proj3a_units(engines):
                # stripe-3 slab-0 (heads 0,1) partial proj into SBUF, so the
                # post-attention tail only runs the slab-1 half
                units = []
                for u_i, (tgl, oi) in enumerate(
                        (tgl, oi) for tgl in range(4) for oi in range(2)):
                    def unit(tgl=tgl, oi=oi, u_i=u_i):
                        po = ps_qkv.tile([128, 512], f32, tag="big", name="po")
                        nc.tensor.matmul(
                            po[:], yTs[3][:, 0, ts(tgl, 128)],
                            wpt_sb[:, 0, ts(oi, 512)], start=True, stop=True)
                        dst = hs[:, tgl, ts(oi, 512)]
                        if engines[u_i % len(engines)] == "act":
                            nc.scalar.activation(dst, po[:], COPY)
                        else:
                            nc.vector.tensor_copy(dst, po[:])
                    units.append((unit, 213))
                return units

            def emit_proj3b(engines):
                otS = op.tile([128, 4, C], bf16, tag="otS", name="otS")
                order = [(tgl, oi) for tgl in range(4) for oi in range(2)]
                for wave in range(2):
                    chunk = order[4 * wave:4 * wave + 4]
                    pos = []
                    # wave's identity-matmuls first: they only need hs, so
                    # they execute during the final norm chain
                    for w_i, (tgl, oi) in enumerate(chunk):
                        if w_i < 2:
                            po = ps_qkv.tile([128, 512], f32, tag="big",
                                             name="po")
                        else:
                            pot = ps_s.tile([128, 2, 512], f32, tag="sc",
                                            name="pot")
                            po = pot[:, 0, :]
                        nc.tensor.matmul(
                            po[:], I_sb[:], hs[:, tgl, ts(oi, 512)],
                            start=True, stop=False)
                        pos.append(po)
                    for w_i, (tgl, oi) in enumerate(chunk):
                        po = pos[w_i]
                        nc.tensor.matmul(
                            po[:], yTs[3][:, 1, ts(tgl, 128)],
                            wpt_sb[:, 1, ts(oi, 512)], start=False, stop=True)
                        dst = otS[:, tgl, ts(oi, 512)]
                        if engines[w_i % len(engines)] == "act":
                            nc.scalar.activation(dst, po[:], COPY)
                        else:
                            nc.vector.tensor_copy(dst, po[:])
                        if w_i % 2 == 1:
                            lo = 2 * wave + w_i // 2
                            nc.sync.dma_start(
                                out_r[:, 12 + lo:13 + lo, :],
                                otS[:, lo:lo + 1, :])

            def interleave(primary, fillers):
                # primary: (unit, act_ns); fillers: (unit, pe_ns).
                # Emit fillers so cumulative filler-PE time tracks cumulative
                # primary-ACT time proportionally.
                ptot = sum(w for _, w in primary) or 1
                ftot = sum(w for _, w in fillers)
                fi, pacc, facc = 0, 0.0, 0.0
                if fillers:
                    fillers[0][0]()
                    facc += fillers[0][1]
                    fi = 1
                for u, w in primary:
                    u()
                    pacc += w
                    while fi < len(fillers) and \
                            facc <= pacc / ptot * ftot:
                        fillers[fi][0]()
                        facc += fillers[fi][1]
                        fi += 1
                while fi < len(fillers):
                    fillers[fi][0]()
                    fi += 1

            # iter 0: QKV stripe 0 alone; prefetch x stripe 1, then wpt
            nc.sync.dma_start(xts[1][:, :, :], xT_r[:, :, ts(1, 512)])
            nc.sync.dma_start(
                wpt_sb[:], wpt.rearrange("(s p) o -> p s o", p=128))
            for u, _w in qkv_units(0):
                u()

            # steady iters: attention i-1 + QKV i (proj is all deferred to
            # the late, otherwise ACT-bound iterations)
            for i in range(1, 4):
                if i < 3:
                    xt_next = xts[(i + 1) % 2]
                    nc.sync.dma_start(xt_next[:, :, :],
                                      xT_r[:, :, ts(i + 1, 512)])
                interleave(attn_units(i - 1), qkv_units(i))
                if i == 3:
                    # stripe-3 head 0 pulled forward (qkv 3 is complete by
                    # late iter 3) so the final iteration stays PE-bound
                    interleave(attn_units(3, heads=[0]),
                               proj_units(0, ["dve"], dma_splits=2))

            # stripe-3 head 1 + proj 1; then heads 2-3 with proj 2 and the
            # slab-0 half of proj 3 (h0/h1 normalized by then) as PE fill
            interleave(attn_units(3, heads=[1]),
                       proj_units(1, ["dve"], dma_splits=2))
            interleave(attn_units(3, heads=[2, 3]),
                       proj_units(2, ["dve"], dma_splits=2)
                       + proj3a_units(["dve"]))
            # tail: only the slab-1 half remains (PE re-injects the slab-0
            # half via an identity matmul); copies fan across idle engines
            emit_proj3b(["act", "dve"])

    nc.compile()
    return nc


def _get_compiled():
    global _COMPILED
    if _COMPILED is None:
        _COMPILED = _build()
    return _COMPILED


def _host_prep(x, W_attn, b_attn, W_proj, b_proj):
    # 1/sqrt(D) split evenly between q and k so both stay in fp8's
    # comfortable range; q/k channel order permuted per 128-row slab to
    # [h0 d0-31, h1 d0-31, h0 d32-63, h1 d32-63] so the device bias-add
    # lands directly in the DoubleRow fp8 layout.
    s8 = np.float32(1.0 / np.sqrt(np.sqrt(np.float32(D))))
    perm = np.concatenate([np.arange(0, 32), np.arange(64, 96),
                           np.arange(32, 64), np.arange(96, 128)])
    b16 = ml_dtypes.bfloat16
    xTb = [np.ascontiguousarray(x[b].T).astype(b16) for b in range(B)]
    Sm = (np.arange(128, dtype=np.int32)[None, :]
          >= np.arange(128, dtype=np.int32)[:, None]).astype(b16)
    Idm = np.eye(128, dtype=np.float32).astype(b16)
    in_maps = []
    for c in range(N_CORES):
        b, g = divmod(c, 4)
        ch = slice(CH * g, CH * (g + 1))
        Wq = (W_attn[ch] * s8).reshape(2, 128, C)[:, perm].reshape(CH, C)
        Wk = (W_attn[C:][ch] * s8).reshape(2, 128, C)[:, perm].reshape(CH, C)
        Wv = W_attn[2 * C:][ch]
        wt_c = np.ascontiguousarray(
            np.concatenate([Wq, Wk, Wv], axis=0).T).astype(b16)
        bq = (b_attn[ch] * s8).reshape(2, 128)[:, perm].reshape(CH)
        bk = (b_attn[C:][ch] * s8).reshape(2, 128)[:, perm].reshape(CH)
        bqk_c = np.ascontiguousarray(
            np.concatenate([bq, bk]).reshape(4, 128).T).astype(np.float32)
        wpt_c = np.ascontiguousarray(W_proj[:, ch].T).astype(b16)
        in_maps.append({
            "xT": xTb[b],
            "wt": wt_c,
            "wpt": wpt_c,
            "bqk": bqk_c,
            "Sm": Sm,
            "Idm": Idm,
        })
    return in_maps


def kernel(x, W_attn, b_attn, W_proj, b_proj):
    x = np.asarray(x, dtype=np.float32)
    W_attn = np.asarray(W_attn, dtype=np.float32)
    b_attn = np.asarray(b_attn, dtype=np.float32)
    W_proj = np.asarray(W_proj, dtype=np.float32)
    b_proj = np.asarray(b_proj, dtype=np.float32)

    nc = _get_compiled()
    in_maps = _host_prep(x, W_attn, b_attn, W_proj, b_proj)
    res = run_bass_kernel_spmd(nc, in_maps, core_ids=list(range(N_CORES)))

    # v-bias passes through softmax exactly: y = y_nobias + b_v, so its
    # output-projection contribution W_proj @ b_v is added here instead
    # of on the device.
    bv = b_attn[2 * C:]
    b_eff = (b_proj + W_proj.astype(np.float64) @ bv.astype(np.float64)
             ).astype(np.float32)

    out = np.empty((B, T, C), dtype=np.float32)
    for b in range(B):
        acc = res.results[4 * b]["out_partial"].astype(np.float32)
        for g in range(1, 4):
            acc += res.results[4 * b + g]["out_partial"].astype(np.float32)
        out[b] = acc + b_eff
    return out
